# revision 10
# baseline (speedup 1.0000x reference)
"""Trainium2 Bass kernel for nn_CRF_SelfAttention_49065706390003.

Math: the reference's MultiheadAttention runs with sequence length 1, so the
softmax is over a singleton axis (all ones) and ctx == v; the per-scale
multiply-by-counts / divide-by-counts cancels, so the whole module collapses to

    out[p, f, :] = 0.75 * (emb[f, p, :] @ Wv.T @ Wo.T) @ Wmp.T + b_eff
    b_eff        = 0.75 * Wmp @ (Wo @ bv + bo) + bmp

i.e. one skinny linear map x @ G + b_eff with G = 0.75*(Wmp @ Wo @ Wv).T
([2048, 64]).  Wq/Wk/bq/bk are mathematically dead.

Sharding: data-parallel over the n_partitions axis (1024 -> 128 per core);
the small weight matrices are replicated and the fold G = 0.75*(Wmp@Wo@Wv).T
is computed on-device on every core (two chained matmuls + PE transposes),
then each core runs its [2304, 2048] x [2048, 64] token matmul.
"""

import os
import sys

for _p in ("/opt/trn_rl_repo",):
    if _p not in sys.path and os.path.isdir(_p):
        sys.path.insert(0, _p)

from contextlib import ExitStack

import numpy as np

import concourse.tile as tile
from concourse import bacc, mybir
from concourse.bass import ds, ts
from concourse.bass_utils import run_bass_kernel_spmd
from concourse.masks import make_identity

F = 18        # n_frames
PTOT = 1024   # n_partitions
E = 2048      # n_hidden
C = 64        # n_cluster
NCORES = 8
PSH = PTOT // NCORES          # 128 partitions per core
NTOK = F * PSH                # 2304 tokens per core
KC = E // 128                 # 16 contraction chunks
NT = (NTOK + 511) // 512      # 5 token tiles (4x512 + 256)
F32 = mybir.dt.float32

# True: the weight fold G = 0.75*(Wmp@Wo@Wv).T runs on-device (replicated on
# every core). False: fold on host, device only runs the token matmul.
FOLD_ON_DEVICE = True


def _build(fold_on_device: bool):
    nc = bacc.Bacc(
        "TRN2", target_bir_lowering=False, debug=False, num_devices=NCORES
    )
    xT = nc.dram_tensor("xT", [E, NTOK], F32, kind="ExternalInput").ap()
    outT = nc.dram_tensor("outT", [C, NTOK], F32, kind="ExternalOutput").ap()
    if fold_on_device:
        wo = nc.dram_tensor("wo", [E, E], F32, kind="ExternalInput").ap()
        wv = nc.dram_tensor("wv", [E, E], F32, kind="ExternalInput").ap()
        # Wmp.T packed: [128, KC*C], (p, k*C + c) = Wmp[c, k*128 + p]
        wmpT = nc.dram_tensor("wmpT", [128, KC * C], F32, kind="ExternalInput").ap()
        bo_p = nc.dram_tensor("bo_p", [128, KC], F32, kind="ExternalInput").ap()
        bv_p = nc.dram_tensor("bv_p", [128, KC], F32, kind="ExternalInput").ap()
        bmp_p = nc.dram_tensor("bmp_p", [C, 1], F32, kind="ExternalInput").ap()
    else:
        # G packed like wmpT: (p, k*C + c) = G[k*128 + p, c]
        gT = nc.dram_tensor("gT", [128, KC * C], F32, kind="ExternalInput").ap()
        beff_in = nc.dram_tensor("beff", [C, 1], F32, kind="ExternalInput").ap()

    with tile.TileContext(nc) as tc:
        with ExitStack() as ctx:
            consts = ctx.enter_context(tc.tile_pool(name="consts", bufs=1))
            wpool = ctx.enter_context(tc.tile_pool(name="wpool", bufs=3))
            pacc = ctx.enter_context(
                tc.tile_pool(name="pacc", bufs=5, space="PSUM")
            )

            b_eff = consts.tile([C, 1], F32)
            out_sb = consts.tile([C, NTOK], F32)

            if fold_on_device:
                ptr_pool = ctx.enter_context(
                    tc.tile_pool(name="ptr", bufs=2, space="PSUM")
                )
                pb_pool = ctx.enter_context(
                    tc.tile_pool(name="pb", bufs=1, space="PSUM")
                )
                identity = consts.tile([64, 64], F32)
                make_identity(nc, identity)
                wmpT_sb = consts.tile([128, KC * C], F32)
                nc.sync.dma_start(wmpT_sb, wmpT)
                bo_sb = consts.tile([128, KC], F32)
                nc.sync.dma_start(bo_sb, bo_p)
                bv_sb = consts.tile([128, KC], F32)
                nc.sync.dma_start(bv_sb, bv_p)
                bmp_sb = consts.tile([C, 1], F32)
                nc.sync.dma_start(bmp_sb, bmp_p)
                T_sb = consts.tile([C, E], F32)
                Tt_sb = consts.tile([128, KC * C], F32)
                M_sb = consts.tile([C, E], F32)
                Gt_sb = consts.tile([128, KC * C], F32)

                # Column-group packing: even tiles run on PE cols 0-63
                # (psum partitions 0:64), odd tiles on cols 64-127 (psum
                # partitions 64:128) — concurrent streams, separate banks.
                def half(bank, n, w=512):
                    return bank[0:64, :w] if n % 2 == 0 else bank[64:128, :w]

                def tpos(n):
                    return (0, 0) if n % 2 == 0 else (0, 64)

                # Column-group packing: tile n, C-half ch runs on PE cols
                # q*32:(q+1)*32 with q = (n%2)*2+ch, landing in psum
                # partitions q*32:(q+1)*32 — 4 concurrent streams; even n
                # results complete in partitions 0:64, odd n in 64:128.
                def half(bank, n, w=512):
                    return bank[0:64, :w] if n % 2 == 0 else bank[64:128, :w]

                # ---- stage B: T = Wmp @ Wo  -> T_sb [64, 2048]
                pT = [pacc.tile([128, 512], F32, tag="acc", name=f"pT{n}") for n in range(4)]
                for k in range(KC):
                    w_sb = wpool.tile([128, E], F32, tag="w")
                    nc.sync.dma_start(w_sb, wo[ts(k, 128), :])
                    for n in range(4):
                        for ch in range(2):
                            q = (n % 2) * 2 + ch
                            nc.tensor.matmul(
                                pT[n][ds(q * 32, 32), :],
                                wmpT_sb[:, ds(k * C + ch * 32, 32)],
                                w_sb[:, ts(n, 512)],
                                start=(k == 0), stop=(k == KC - 1),
                                tile_position=(0, q * 32),
                            )
                for n in range(4):
                    nc.vector.tensor_copy(T_sb[:, ts(n, 512)], half(pT[n], n))

                # ---- stage B2: Tt = T.T (PE transpose, 16 blocks of [64,128])
                for k in range(KC):
                    ptr = ptr_pool.tile([128, C], F32, tag="tr")
                    nc.tensor.transpose(ptr, T_sb[:, ts(k, 128)], identity)
                    nc.vector.tensor_copy(Tt_sb[:, ts(k, C)], ptr)

                # ---- stage C: M = T @ Wv; bias = Wmp@bo + T@bv
                pM = [pacc.tile([128, 512], F32, tag="acc", name=f"pM{n}") for n in range(4)]
                pb = pb_pool.tile([C, 1], F32)
                for k in range(KC):
                    w_sb = wpool.tile([128, E], F32, tag="w")
                    nc.sync.dma_start(w_sb, wv[ts(k, 128), :])
                    for n in range(4):
                        for ch in range(2):
                            q = (n % 2) * 2 + ch
                            nc.tensor.matmul(
                                pM[n][ds(q * 32, 32), :],
                                Tt_sb[:, ds(k * C + ch * 32, 32)],
                                w_sb[:, ts(n, 512)],
                                start=(k == 0), stop=(k == KC - 1),
                                tile_position=(0, q * 32),
                            )
                    nc.tensor.matmul(
                        pb, wmpT_sb[:, ts(k, C)], bo_sb[:, ds(k, 1)],
                        start=(k == 0), stop=False, tile_position=(0, 0),
                    )
                    nc.tensor.matmul(
                        pb, Tt_sb[:, ts(k, C)], bv_sb[:, ds(k, 1)],
                        start=False, stop=(k == KC - 1), tile_position=(0, 0),
                    )
                for n in range(4):
                    nc.vector.tensor_copy(M_sb[:, ts(n, 512)], half(pM[n], n))
                # b_eff = 0.75 * pb + bmp
                nc.vector.tensor_scalar(
                    out=b_eff, in0=pb, scalar1=0.75, scalar2=bmp_sb,
                    op0=mybir.AluOpType.mult, op1=mybir.AluOpType.add,
                )

                # ---- stage C2: Gt = 0.75 * M.T
                for k in range(KC):
                    ptr = ptr_pool.tile([128, C], F32, tag="tr")
                    nc.tensor.transpose(ptr, M_sb[:, ts(k, 128)], identity)
                    nc.vector.tensor_scalar_mul(Gt_sb[:, ts(k, C)], ptr, 0.75)
            else:
                Gt_sb = consts.tile([128, KC * C], F32)
                nc.sync.dma_start(Gt_sb, gT)
                nc.sync.dma_start(b_eff, beff_in)

            # ---- stage D: outT = G.T @ x (+ b_eff)
            # 4 concurrent PE col-group streams: (token-tile parity) x
            # (C-half). Token tile j's full [64, jw] result lands in psum
            # partitions 0:64 (even j) or 64:128 (odd j) of its own bank.
            def dhalf(bank, j, w):
                return bank[0:64, :w] if j % 2 == 0 else bank[64:128, :w]

            po = [
                pacc.tile([128, 512], F32, tag="acc", name=f"po{j}")
                for j in range(NT)
            ]
            for k in range(KC):
                x_sb = wpool.tile([128, NTOK], F32, tag="x", bufs=6, name="x_sb")
                nc.sync.dma_start(x_sb, xT[ts(k, 128), :])
                for j in range(NT):
                    jw = min(512, NTOK - j * 512)
                    for ch in range(2):
                        q = (j % 2) * 2 + ch
                        nc.tensor.matmul(
                            po[j][ds(q * 32, 32), :jw],
                            Gt_sb[:, ds(k * C + ch * 32, 32)],
                            x_sb[:, ds(j * 512, jw)],
                            start=(k == 0), stop=(k == KC - 1),
                            tile_position=(0, q * 32),
                        )
            for j in range(NT):
                jw = min(512, NTOK - j * 512)
                nc.vector.tensor_scalar_add(
                    out_sb[:, ds(j * 512, jw)], dhalf(po[j], j, jw), b_eff
                )
            nc.sync.dma_start(outT, out_sb)

    nc.compile()
    return nc


_NC_CACHE: dict = {}


def _get_nc(fold_on_device: bool):
    key = fold_on_device
    if key not in _NC_CACHE:
        _NC_CACHE[key] = _build(fold_on_device)
    return _NC_CACHE[key]


def _pack_kpc(a: np.ndarray) -> np.ndarray:
    """[KC*128, C] -> [128, KC*C] with (p, k*C+c) = a[k*128+p, c]."""
    return np.ascontiguousarray(
        a.reshape(KC, 128, C).transpose(1, 0, 2).reshape(128, KC * C)
    )


def make_in_maps(inputs: dict, fold_on_device: bool):
    emb = np.ascontiguousarray(np.asarray(inputs["emb"], np.float32))
    Wv = np.ascontiguousarray(np.asarray(inputs["Wv"], np.float32))
    Wo = np.ascontiguousarray(np.asarray(inputs["Wo"], np.float32))
    Wmp = np.ascontiguousarray(np.asarray(inputs["Wmp"], np.float32))
    bv = np.asarray(inputs["bv"], np.float32)
    bo = np.asarray(inputs["bo"], np.float32)
    bmp = np.asarray(inputs["bmp"], np.float32)

    if fold_on_device:
        shared = {
            "wo": Wo,
            "wv": Wv,
            # Wmp.T packed: (p, k*C+c) = Wmp[c, k*128+p]
            "wmpT": np.ascontiguousarray(
                Wmp.reshape(C, KC, 128).transpose(2, 1, 0).reshape(128, KC * C)
            ),
            "bo_p": np.ascontiguousarray(bo.reshape(KC, 128).T),
            "bv_p": np.ascontiguousarray(bv.reshape(KC, 128).T),
            "bmp_p": np.ascontiguousarray(bmp[:, None]),
        }
    else:
        T = Wmp @ Wo
        G = 0.75 * (T @ Wv).T
        beff = 0.75 * (Wmp @ (Wo @ bv + bo)) + bmp
        shared = {
            "gT": _pack_kpc(G.astype(np.float32)),
            "beff": np.ascontiguousarray(beff.astype(np.float32)[:, None]),
        }

    in_maps = []
    for c in range(NCORES):
        sl = emb[:, c * PSH:(c + 1) * PSH, :].reshape(NTOK, E)
        in_maps.append({"xT": np.ascontiguousarray(sl.T), **shared})
    return in_maps


def assemble(results) -> np.ndarray:
    parts = []
    for c in range(NCORES):
        o = np.asarray(results[c]["outT"])  # [C, NTOK]
        parts.append(o.T.reshape(F, PSH, C).transpose(1, 0, 2))
    return np.ascontiguousarray(np.concatenate(parts, axis=0))


def run(inputs: dict, fold_on_device: bool = FOLD_ON_DEVICE, **kw):
    nc = _get_nc(fold_on_device)
    in_maps = make_in_maps(inputs, fold_on_device)
    res = run_bass_kernel_spmd(nc, in_maps, list(range(NCORES)), **kw)
    return assemble(res.results), res


def kernel(**inputs) -> np.ndarray:
    out, _ = run(inputs)
    return out


# revision 13
# speedup vs baseline: 2.5785x; 2.5785x over previous
"""Trainium2 Bass kernel for nn_CRF_SelfAttention_49065706390003.

Math: the reference's MultiheadAttention runs with sequence length 1, so the
softmax is over a singleton axis (all ones) and ctx == v; the per-scale
multiply-by-counts / divide-by-counts cancels, so the whole module collapses to

    out[p, f, :] = 0.75 * (emb[f, p, :] @ Wv.T @ Wo.T) @ Wmp.T + b_eff
    b_eff        = 0.75 * Wmp @ (Wo @ bv + bo) + bmp

i.e. one skinny linear map x @ G + b_eff with G = 0.75*(Wmp @ Wo @ Wv).T
([2048, 64]).  Wq/Wk/bq/bk are mathematically dead.

Sharding: data-parallel over the n_partitions axis (1024 -> 128 per core);
the small weight matrices are replicated and the fold G = 0.75*(Wmp@Wo@Wv).T
is computed on-device on every core (two chained matmuls + PE transposes),
then each core runs its [2304, 2048] x [2048, 64] token matmul.
"""

import os
import sys

for _p in ("/opt/trn_rl_repo",):
    if _p not in sys.path and os.path.isdir(_p):
        sys.path.insert(0, _p)

from contextlib import ExitStack

import numpy as np

import concourse.tile as tile
from concourse import bacc, mybir
from concourse.bass import ds, ts
from concourse.bass_utils import run_bass_kernel_spmd
from concourse.masks import make_identity

F = 18        # n_frames
PTOT = 1024   # n_partitions
E = 2048      # n_hidden
C = 64        # n_cluster
NCORES = 8
PSH = PTOT // NCORES          # 128 partitions per core
NTOK = F * PSH                # 2304 tokens per core
KC = E // 128                 # 16 contraction chunks
NT = (NTOK + 511) // 512      # 5 token tiles (4x512 + 256)
F32 = mybir.dt.float32

# True: the weight fold G = 0.75*(Wmp@Wo@Wv).T runs on-device (replicated on
# every core). False: fold on host, device only runs the token matmul.
FOLD_ON_DEVICE = True


def _build(fold_on_device: bool):
    nc = bacc.Bacc(
        "TRN2", target_bir_lowering=False, debug=False, num_devices=NCORES
    )
    xT = nc.dram_tensor("xT", [E, NTOK], F32, kind="ExternalInput").ap()
    outT = nc.dram_tensor("outT", [C, NTOK], F32, kind="ExternalOutput").ap()
    if fold_on_device:
        wo = nc.dram_tensor("wo", [E, E], F32, kind="ExternalInput").ap()
        wv = nc.dram_tensor("wv", [E, E], F32, kind="ExternalInput").ap()
        # Wmp.T packed: [128, KC*C], (p, k*C + c) = Wmp[c, k*128 + p]
        wmpT = nc.dram_tensor("wmpT", [128, KC * C], F32, kind="ExternalInput").ap()
        bo_p = nc.dram_tensor("bo_p", [128, KC], F32, kind="ExternalInput").ap()
        bv_p = nc.dram_tensor("bv_p", [128, KC], F32, kind="ExternalInput").ap()
        bmp_p = nc.dram_tensor("bmp_p", [C, 1], F32, kind="ExternalInput").ap()
    else:
        # G packed like wmpT: (p, k*C + c) = G[k*128 + p, c]
        gT = nc.dram_tensor("gT", [128, KC * C], F32, kind="ExternalInput").ap()
        beff_in = nc.dram_tensor("beff", [C, 1], F32, kind="ExternalInput").ap()

    with tile.TileContext(nc) as tc:
        with ExitStack() as ctx:
            consts = ctx.enter_context(tc.tile_pool(name="consts", bufs=1))
            wpool = ctx.enter_context(tc.tile_pool(name="wpool", bufs=3))
            pacc = ctx.enter_context(
                tc.tile_pool(name="pacc", bufs=5, space="PSUM")
            )

            b_eff = consts.tile([C, 1], F32)
            out_sb = consts.tile([C, NTOK], F32)

            if fold_on_device:
                ptr_pool = ctx.enter_context(
                    tc.tile_pool(name="ptr", bufs=2, space="PSUM")
                )
                pb_pool = ctx.enter_context(
                    tc.tile_pool(name="pb", bufs=1, space="PSUM")
                )
                identity = consts.tile([64, 64], F32)
                make_identity(nc, identity)
                wmpT_sb = consts.tile([128, KC * C], F32)
                nc.sync.dma_start(wmpT_sb, wmpT)
                bo_sb = consts.tile([128, KC], F32)
                nc.sync.dma_start(bo_sb, bo_p)
                bv_sb = consts.tile([128, KC], F32)
                nc.sync.dma_start(bv_sb, bv_p)
                bmp_sb = consts.tile([C, 1], F32)
                nc.sync.dma_start(bmp_sb, bmp_p)
                T_sb = consts.tile([C, E], F32)
                Tt_sb = consts.tile([128, KC * C], F32)
                M_sb = consts.tile([C, E], F32)
                Gt_sb = consts.tile([128, KC * C], F32)

                # Column-group packing: even tiles run on PE cols 0-63
                # (psum partitions 0:64), odd tiles on cols 64-127 (psum
                # partitions 64:128) — concurrent streams, separate banks.
                def half(bank, n, w=512):
                    return bank[0:64, :w] if n % 2 == 0 else bank[64:128, :w]

                def tpos(n):
                    return (0, 0) if n % 2 == 0 else (0, 64)

                # Column-group packing: tile n, C-half ch runs on PE cols
                # q*32:(q+1)*32 with q = (n%2)*2+ch, landing in psum
                # partitions q*32:(q+1)*32 — 4 concurrent streams; even n
                # results complete in partitions 0:64, odd n in 64:128.
                def half(bank, n, w=512):
                    return bank[0:64, :w] if n % 2 == 0 else bank[64:128, :w]

                # ---- stage B: T = Wmp @ Wo  -> T_sb [64, 2048]
                pT = [pacc.tile([128, 512], F32, tag="acc", name=f"pT{n}") for n in range(4)]
                for k in range(KC):
                    w_sb = wpool.tile([128, E], F32, tag="w")
                    nc.sync.dma_start(w_sb, wo[ts(k, 128), :])
                    lh = wmpT_sb[:, ts(k, C)]
                    for n in range(4):
                        nc.tensor.matmul(
                            half(pT[n], n), lh, w_sb[:, ts(n, 512)],
                            start=(k == 0), stop=(k == KC - 1),
                            tile_position=(0, 0) if n % 2 == 0 else (0, 64),
                        )
                for n in range(4):
                    nc.vector.tensor_copy(T_sb[:, ts(n, 512)], half(pT[n], n))

                # ---- stage B2: Tt = T.T (PE transpose, 16 blocks of [64,128])
                for k in range(KC):
                    ptr = ptr_pool.tile([128, C], F32, tag="tr")
                    nc.tensor.transpose(ptr, T_sb[:, ts(k, 128)], identity)
                    nc.vector.tensor_copy(Tt_sb[:, ts(k, C)], ptr)

                # ---- stage C: M = T @ Wv; bias = Wmp@bo + T@bv
                pM = [pacc.tile([128, 512], F32, tag="acc", name=f"pM{n}") for n in range(4)]
                pb = pb_pool.tile([C, 1], F32)
                for k in range(KC):
                    w_sb = wpool.tile([128, E], F32, tag="w")
                    nc.sync.dma_start(w_sb, wv[ts(k, 128), :])
                    lh = Tt_sb[:, ts(k, C)]
                    for n in range(4):
                        nc.tensor.matmul(
                            half(pM[n], n), lh, w_sb[:, ts(n, 512)],
                            start=(k == 0), stop=(k == KC - 1),
                            tile_position=(0, 0) if n % 2 == 0 else (0, 64),
                        )
                    nc.tensor.matmul(
                        pb, wmpT_sb[:, ts(k, C)], bo_sb[:, ds(k, 1)],
                        start=(k == 0), stop=False, tile_position=(0, 0),
                    )
                    nc.tensor.matmul(
                        pb, Tt_sb[:, ts(k, C)], bv_sb[:, ds(k, 1)],
                        start=False, stop=(k == KC - 1), tile_position=(0, 0),
                    )
                for n in range(4):
                    nc.vector.tensor_copy(M_sb[:, ts(n, 512)], half(pM[n], n))
                # b_eff = 0.75 * pb + bmp
                nc.vector.tensor_scalar(
                    out=b_eff, in0=pb, scalar1=0.75, scalar2=bmp_sb,
                    op0=mybir.AluOpType.mult, op1=mybir.AluOpType.add,
                )

                # ---- stage C2: Gt = 0.75 * M.T
                for k in range(KC):
                    ptr = ptr_pool.tile([128, C], F32, tag="tr")
                    nc.tensor.transpose(ptr, M_sb[:, ts(k, 128)], identity)
                    nc.vector.tensor_scalar_mul(Gt_sb[:, ts(k, C)], ptr, 0.75)
            else:
                Gt_sb = consts.tile([128, KC * C], F32)
                nc.sync.dma_start(Gt_sb, gT)
                nc.sync.dma_start(b_eff, beff_in)

            # ---- stage D: outT = G.T @ x (+ b_eff)
            # 4 concurrent PE col-group streams: (token-tile parity) x
            # (C-half). Token tile j's full [64, jw] result lands in psum
            # partitions 0:64 (even j) or 64:128 (odd j) of its own bank.
            def dhalf(bank, j, w):
                return bank[0:64, :w] if j % 2 == 0 else bank[64:128, :w]

            po = [
                pacc.tile([128, 512], F32, tag="acc", name=f"po{j}")
                for j in range(NT)
            ]
            for k in range(KC):
                x_sb = wpool.tile([128, NTOK], F32, tag="x", bufs=6, name="x_sb")
                nc.sync.dma_start(x_sb, xT[ts(k, 128), :])
                lh = Gt_sb[:, ts(k, C)]
                for j in range(NT):
                    jw = min(512, NTOK - j * 512)
                    nc.tensor.matmul(
                        dhalf(po[j], j, jw), lh, x_sb[:, ds(j * 512, jw)],
                        start=(k == 0), stop=(k == KC - 1),
                        tile_position=(0, 0) if j % 2 == 0 else (0, 64),
                    )
            for j in range(NT):
                jw = min(512, NTOK - j * 512)
                nc.vector.tensor_scalar_add(
                    out_sb[:, ds(j * 512, jw)], dhalf(po[j], j, jw), b_eff
                )
            nc.sync.dma_start(outT, out_sb)

    nc.compile()
    return nc


_NC_CACHE: dict = {}


def _get_nc(fold_on_device: bool):
    key = fold_on_device
    if key not in _NC_CACHE:
        _NC_CACHE[key] = _build(fold_on_device)
    return _NC_CACHE[key]


def _pack_kpc(a: np.ndarray) -> np.ndarray:
    """[KC*128, C] -> [128, KC*C] with (p, k*C+c) = a[k*128+p, c]."""
    return np.ascontiguousarray(
        a.reshape(KC, 128, C).transpose(1, 0, 2).reshape(128, KC * C)
    )


def make_in_maps(inputs: dict, fold_on_device: bool):
    emb = np.ascontiguousarray(np.asarray(inputs["emb"], np.float32))
    Wv = np.ascontiguousarray(np.asarray(inputs["Wv"], np.float32))
    Wo = np.ascontiguousarray(np.asarray(inputs["Wo"], np.float32))
    Wmp = np.ascontiguousarray(np.asarray(inputs["Wmp"], np.float32))
    bv = np.asarray(inputs["bv"], np.float32)
    bo = np.asarray(inputs["bo"], np.float32)
    bmp = np.asarray(inputs["bmp"], np.float32)

    if fold_on_device:
        shared = {
            "wo": Wo,
            "wv": Wv,
            # Wmp.T packed: (p, k*C+c) = Wmp[c, k*128+p]
            "wmpT": np.ascontiguousarray(
                Wmp.reshape(C, KC, 128).transpose(2, 1, 0).reshape(128, KC * C)
            ),
            "bo_p": np.ascontiguousarray(bo.reshape(KC, 128).T),
            "bv_p": np.ascontiguousarray(bv.reshape(KC, 128).T),
            "bmp_p": np.ascontiguousarray(bmp[:, None]),
        }
    else:
        T = Wmp @ Wo
        G = 0.75 * (T @ Wv).T
        beff = 0.75 * (Wmp @ (Wo @ bv + bo)) + bmp
        shared = {
            "gT": _pack_kpc(G.astype(np.float32)),
            "beff": np.ascontiguousarray(beff.astype(np.float32)[:, None]),
        }

    in_maps = []
    for c in range(NCORES):
        sl = emb[:, c * PSH:(c + 1) * PSH, :].reshape(NTOK, E)
        in_maps.append({"xT": np.ascontiguousarray(sl.T), **shared})
    return in_maps


def assemble(results) -> np.ndarray:
    parts = []
    for c in range(NCORES):
        o = np.asarray(results[c]["outT"])  # [C, NTOK]
        parts.append(o.T.reshape(F, PSH, C).transpose(1, 0, 2))
    return np.ascontiguousarray(np.concatenate(parts, axis=0))


def run(inputs: dict, fold_on_device: bool = FOLD_ON_DEVICE, **kw):
    nc = _get_nc(fold_on_device)
    in_maps = make_in_maps(inputs, fold_on_device)
    res = run_bass_kernel_spmd(nc, in_maps, list(range(NCORES)), **kw)
    return assemble(res.results), res


def kernel(**inputs) -> np.ndarray:
    out, _ = run(inputs)
    return out
